# revision 21
# baseline (speedup 1.0000x reference)
"""Fused multi-head attention forward (B=2, S=2048, SIZE=1024, H=16) on 8
Trainium2 NeuronCores.

Sharding: 2-way data parallel over batch x 4-way tensor parallel over heads
(Megatron style). Each core computes 4 heads of one batch element end-to-end
(QKV projections for its 256-dim slice, attention, and a partial output
projection); the host sums the 4 partials per batch and adds the output
bias. The value bias drops out of attention algebraically (softmax rows sum
to 1) so the host folds `bv @ Wo.T` into a constant row; the key bias drops
out too (it only shifts every score for a given query by the same constant,
which softmax ignores), so `bk` is never sent to the device.

The kernel is organized around keeping the Scalar (ACT) engine -- the only
engine that can evaluate exp -- saturated, since its ~16.8M exps/core are
the critical resource. Everything else (QKV projections, score and context
matmuls, softmax denominators, output projection) is PE/DVE work scheduled
into the ACT-bound slack:

  - K and Q arrive as column chunks so the first score matmul (and hence the
    first exp) can issue within a few microseconds.
  - Scores for a (head-pair, q-chunk, key-tile) land in a double-buffered
    [128,1024] fp32 PSUM tile (key positions on partitions, 2 heads x 512 q
    on free); one FD=1024 ACTIVATE turns it into bf16 exp values in SBUF.
  - Context accumulates in a [128,512] PSUM tile (2 heads stacked at
    partitions 0-63 / 64-127, giving col-group concurrency) over the 16
    key tiles; score matmuls use the 2 heads' 64-dim row groups (0-63 /
    64-127) for row-group concurrency.
  - Softmax denominators: exp tiles are summed in a two-level tree on DVE
    (pairwise, then pairs-of-pairs), and a deferred burst of M=2
    ones-selector matmuls accumulates per-query column sums into a [2,512]
    PSUM tile; the reciprocal runs on a [64,16] spread (DVE reciprocal is
    8 cyc/elem, so the spread is ~20x cheaper than [2,512]) reached via a
    DRAM round-trip that also broadcasts the per-query reciprocals across
    the head-dim partitions for the normalize.
  - The output projection consumes normalized ctx (bf16) as 2-matmul
    chains per 128q x 512o tile, with bf16 output tiles (the host sums the
    tensor-parallel partials in fp32).

Priorities: score matmuls are strongly boosted (they feed the exp stream),
ctx matmuls moderately (a lagging ctx pins exp-tile buffers and stalls the
exp stream), the exp-sum adds and deferred denominator bursts in between
(they release those same buffers). Projection chains, output projection,
and normalize chains run as fillers in the ACT-bound slack.
"""

import numpy as np
import ml_dtypes

import concourse.bass as bass
import concourse.tile as tile
from concourse import bacc, mybir
from concourse.bass_utils import run_bass_kernel_spmd

B, S, SIZE, H, D = 2, 2048, 1024, 16, 64
NCORES = 8
HGROUPS = 4                # tensor-parallel head groups
H_LOC = H // HGROUPS       # 4 heads per core
D_LOC = H_LOC * D          # 256 projection dims per core
MT = D_LOC // 128          # 2 head-pairs per core
ET = SIZE // 128           # 8 contraction tiles for projections
KT = S // 128              # 16 key tiles of 128
QC = S // 512              # 4 query chunks of 512

_NC = None


def build():
    global _NC
    if _NC is not None:
        return _NC
    f32, bf16 = mybir.dt.float32, mybir.dt.bfloat16
    Exp = mybir.ActivationFunctionType.Exp

    nc = bacc.Bacc("TRN2", target_bir_lowering=False, debug=False)
    # inputs arrive pre-tiled so every DMA reads contiguous 1-8KB
    # per-partition lines (strided column-chunk reads ran at ~250GB/s)
    qTq_d = nc.dram_tensor("qTq", [QC, 128, ET, 512], bf16, kind="ExternalInput").ap()
    kTq_d = nc.dram_tensor("kTq", [QC, 128, ET, 512], bf16, kind="ExternalInput").ap()
    vTq_d = nc.dram_tensor("vTq", [KT, 128, ET, 128], bf16, kind="ExternalInput").ap()
    WqT_d = nc.dram_tensor("WqT", [SIZE, D_LOC], bf16, kind="ExternalInput").ap()
    WkT_d = nc.dram_tensor("WkT", [SIZE, D_LOC], bf16, kind="ExternalInput").ap()
    WvT_d = nc.dram_tensor("WvT", [SIZE, D_LOC], bf16, kind="ExternalInput").ap()
    WoT_d = nc.dram_tensor("WoT", [D_LOC, SIZE], bf16, kind="ExternalInput").ap()
    bq_d = nc.dram_tensor("bq", [D_LOC], f32, kind="ExternalInput").ap()
    out_d = nc.dram_tensor("out", [S, SIZE], bf16, kind="ExternalOutput").ap()

    with tile.TileContext(nc) as tc:
        with (
            tc.tile_pool(name="persist", bufs=1) as persist,
            tc.tile_pool(name="kx", bufs=2) as kxp,
            tc.tile_pool(name="qx", bufs=2) as qxp,
            tc.tile_pool(name="vx", bufs=8) as vxp,
            tc.tile_pool(name="esb", bufs=19) as esb,
            tc.tile_pool(name="e2p", bufs=10) as e2p,
            tc.tile_pool(name="e4p", bufs=8) as e4p,
            tc.tile_pool(name="small", bufs=2) as small,
            tc.tile_pool(name="osb", bufs=4) as osb,
            tc.tile_pool(name="psS", bufs=2, space="PSUM") as psS,
            tc.tile_pool(name="psC", bufs=2, space="PSUM") as psC,
            tc.tile_pool(name="psP", bufs=2, space="PSUM") as psP,
            tc.tile_pool(name="dscr", bufs=2, space="DRAM") as dscr,
        ):
            # ---- persistent weights / activations ----
            wk_sb = persist.tile([128, ET, D_LOC], bf16)
            wq_sb = persist.tile([128, ET, D_LOC], bf16)
            wv_sb = persist.tile([128, ET, D_LOC], bf16)
            wo_sb = persist.tile([128, MT, SIZE], bf16)
            bq_sb = persist.tile([128, MT], f32)
            qh_sb = persist.tile([128, MT, S], bf16)   # [dim-in-pair, pair, q]
            kh_sb = persist.tile([128, MT, S], bf16)   # [dim-in-pair, pair, k]
            vh_sb = persist.tile([128, KT, H_LOC, D], bf16)  # [k%128, kt, head, d]
            ctx_sb = persist.tile([128, MT, S], bf16)  # normalized ctxT

            nc.sync.dma_start(wk_sb[:], WkT_d.rearrange("(et p) m -> p et m", p=128))

            # ones-selector columns for the denominator matmuls:
            # onesc[hsel] is [128, 2] with column hsel all-ones, other zero,
            # so lhsT=onesc[h] accumulates sum-over-partitions into row h of
            # the [2, 512] denominator tile (and +0 into the other row).
            sel_f = persist.tile([128, 2, 2], f32)
            nc.vector.memset(sel_f[:], 0.0)
            nc.vector.memset(sel_f[:, 0, 0:1], 1.0)
            nc.vector.memset(sel_f[:, 1, 1:2], 1.0)
            sel_bf = persist.tile([128, 2, 2], bf16)
            nc.vector.tensor_copy(sel_bf[:], sel_f[:])
            # prime the ACT exp table while projections run
            warm = persist.tile([128, 1], bf16)
            nc.scalar.activation(warm[:], sel_f[:, 0, 0:1], Exp)

            # ---- K projection, pair 0 first (chunked so scores start early) --
            kxs = [None] * QC

            def k_load(c):
                kx = kxp.tile([128, ET, 512], bf16, tag="kx")
                nc.sync.dma_start(kx[:], kTq_d[c])
                kxs[c] = kx

            def k_chain(c, mt):
                pp = psP.tile([128, 512], f32, tag="pp")
                for et in range(ET):
                    nc.tensor.matmul(
                        pp[:], wk_sb[:, et, mt * 128:(mt + 1) * 128],
                        kxs[c][:, et, :], start=(et == 0), stop=(et == ET - 1))
                nc.vector.tensor_copy(kh_sb[:, mt, c * 512:(c + 1) * 512], pp[:])

            qxs = [None] * QC

            def q_load(qc):
                qx = qxp.tile([128, ET, 512], bf16, tag="qx")
                nc.sync.dma_start(qx[:], qTq_d[qc])
                qxs[qc] = qx

            def q_chain(qc, mt):
                pp = psP.tile([128, 512], f32, tag="pp")
                for et in range(ET):
                    nc.tensor.matmul(
                        pp[:], wq_sb[:, et, mt * 128:(mt + 1) * 128],
                        qxs[qc][:, et, :], start=(et == 0), stop=(et == ET - 1))
                nc.vector.tensor_scalar_add(
                    qh_sb[:, mt, qc * 512:(qc + 1) * 512], pp[:], bq_sb[:, mt:mt + 1])

            # startup is latency-critical: interleave loads and chains so the
            # first score matmul only waits on {wk, kx0, wq, qx0}
            k_load(0)
            k_chain(0, 0)
            nc.sync.dma_start(wq_sb[:], WqT_d.rearrange("(et p) m -> p et m", p=128))
            q_load(0)
            nc.sync.dma_start(bq_sb[:], bq_d.rearrange("(mt p) -> p mt", p=128))
            q_chain(0, 0)
            k_chain(0, 1)
            k_load(1)
            k_chain(1, 0)
            q_chain(0, 1)
            k_chain(1, 1)
            k_load(2)
            k_chain(2, 0)
            k_chain(2, 1)
            k_load(3)
            k_chain(3, 0)
            k_chain(3, 1)
            # PE clock warm-up: the HAM unthrottles only after ~3.4us of
            # sustained matmul activity, so burn that in the initial DMA-wait
            # dead zone on dummy matmuls instead of running the first K
            # chains at half clock. Emitted after the startup chains so real
            # work outranks them; the scores PSUM pool is idle until ~16us.
            zz = persist.tile([128, 512], bf16)
            nc.vector.memset(zz[:], 0.0)
            wp = psS.tile([128, 1024], f32, tag="scs", name="warmup")
            for _ in range(14):
                nc.tensor.matmul(wp[0:16, 0:512], zz[:, 0:16], zz[:],
                                 start=True, stop=True)
            nc.sync.dma_start(wv_sb[:], WvT_d.rearrange("(et p) m -> p et m", p=128))

            # ---- V projection (per key-tile chains; filler for iteration 0) --
            vxs = []
            for st in range(KT):
                vx = vxp.tile([128, ET, 128], bf16, tag="vx")
                nc.sync.dma_start(vx[:], vTq_d[st])
                vxs.append(vx)
            for st in range(KT):
                pp = psP.tile([128, 512], f32, tag="pp")
                for et in range(ET):
                    nc.tensor.matmul(
                        pp[:, 0:D_LOC], vxs[st][:, et, :], wv_sb[:, et, :],
                        start=(et == 0), stop=(et == ET - 1))
                nc.vector.tensor_copy(
                    vh_sb[:, st, :, :],
                    pp[:, 0:D_LOC].rearrange("p (h d) -> p h d", h=H_LOC))

            nc.sync.dma_start(wo_sb[:], WoT_d.rearrange("(hp p) o -> p hp o", p=128))

            # ---- attention iterations: (pair, q-chunk) ----
            iters = [(pr, qc) for qc in range(QC) for pr in range(MT)]
            pending = None  # deferred finisher from the previous iteration

            def dn_burst(dn, e4s, lo, hi_, start, stop):
                # accumulate per-query exp sums of e4s[lo:hi_] into dn [2,512]
                for p in range(lo, hi_):
                    for hi in range(2):
                        nc.tensor.matmul(
                            dn[0:2, :], sel_bf[:, hi, :],
                            e4s[p][:, hi * 512:(hi + 1) * 512],
                            start=(start and p == lo and hi == 0),
                            stop=(stop and p == hi_ - 1 and hi == 1))

            def finish_iteration(pr, qc, dn, ctx_ps, eng=None):
                dma = (eng or nc.sync).dma_start
                # reciprocal of the denominators, spread across 64 partitions
                # via a DRAM round-trip (DVE reciprocal is 8 cyc/elem, so the
                # [2,512] layout would cost ~4.3us; [64,16] costs ~0.2us),
                # then broadcast back across the head-dim partitions.
                sums = small.tile([2, 512], f32, tag="sums")
                nc.vector.tensor_copy(sums[:], dn[0:2, :])
                scr = dscr.tile([1024], f32, tag="scr")
                dma(scr[:].rearrange("(p x) -> p x", p=2), sums[:])
                spread = small.tile([64, 16], f32, tag="spread")
                dma(spread[:], scr[:].rearrange("(p x) -> p x", p=64))
                spread_r = small.tile([64, 16], f32, tag="spreadr")
                nc.vector.reciprocal(spread_r[:], spread[:])
                scr2 = dscr.tile([1024], f32, tag="scr2")
                dma(scr2[:].rearrange("(p x) -> p x", p=64), spread_r[:])
                brec = small.tile([128, 512], f32, tag="brec")
                for hi in range(2):
                    part = scr2[hi * 512:(hi + 1) * 512]
                    dma(brec[hi * 64:(hi + 1) * 64, :],
                        bass.AP(tensor=part.tensor, offset=part.offset,
                                ap=[[0, 64]] + list(part.ap)))
                # normalize + evacuate ctx accumulator (f32 psum -> bf16 sbuf)
                for hi in range(2):
                    nc.vector.tensor_mul(
                        ctx_sb[hi * 64:(hi + 1) * 64, pr,
                               qc * 512:(qc + 1) * 512],
                        ctx_ps[hi * 64:(hi + 1) * 64, :],
                        brec[hi * 64:(hi + 1) * 64, :])

            def out_proj(qc):
                for sti in range(4):
                    st = qc * 4 + sti
                    for ot in range(2):
                        pp = psP.tile([128, 512], f32, tag="pp")
                        for hp in range(MT):
                            nc.tensor.matmul(
                                pp[:], ctx_sb[:, hp, st * 128:(st + 1) * 128],
                                wo_sb[:, hp, ot * 512:(ot + 1) * 512],
                                start=(hp == 0), stop=(hp == MT - 1))
                        o_sb = osb.tile([128, 512], bf16, tag="o")
                        nc.vector.tensor_copy(o_sb[:], pp[:])
                        nc.sync.dma_start(
                            out_d[st * 128:(st + 1) * 128,
                                  ot * 512:(ot + 1) * 512], o_sb[:])

            for it, (pr, qc) in enumerate(iters):
                if pr == 1 and qc + 1 < QC:
                    # Q projection for the next q-chunk, emitted one iteration
                    # early so it soaks up PE slack before it is needed
                    q_load(qc + 1)
                    q_chain(qc + 1, 0)
                    q_chain(qc + 1, 1)
                last = it == len(iters) - 1
                ctx_ps = psC.tile([128, 512], f32, tag="ctx", name=f"ctx{it}")
                e4s = []
                e_prev = None
                e2_prev = None
                dn = None
                for kt in range(KT):
                    scs = psS.tile([128, 1024], f32, tag="scs")
                    with tc.high_priority(offset=1_000_000):
                        # scores must always outrank filler PE work: they
                        # feed the ACT exp stream, the critical resource
                        for hi in range(2):
                            po = hi * D
                            nc.tensor.matmul(
                                scs[:, hi * 512:(hi + 1) * 512],
                                kh_sb[po:po + D, pr, kt * 128:(kt + 1) * 128],
                                qh_sb[po:po + D, pr, qc * 512:(qc + 1) * 512],
                                start=True, stop=True)
                    e_sb = esb.tile([128, 1024], bf16, tag="e")
                    nc.scalar.activation(e_sb[:], scs[:], Exp)
                    with tc.high_priority(offset=600_000):
                        # ctx outranks filler work so the e-tile ring drains
                        # promptly (a lagging ctx stalls the exp stream)
                        for hi in range(2):
                            head = pr * 2 + hi
                            nc.tensor.matmul(
                                ctx_ps[hi * 64:(hi + 1) * 64, :],
                                vh_sb[:, kt, head, :],
                                e_sb[:, hi * 512:(hi + 1) * 512],
                                start=(kt == 0), stop=(kt == KT - 1))
                    if kt % 2 == 1:
                        # two-level exp-sum tree (all DVE): halves the PE
                        # denominator matmuls; boosted because e/e2 slot
                        # release gates the exp stream
                        with tc.high_priority(offset=500_000):
                            e2 = e2p.tile([128, 1024], bf16, tag="e2")
                            nc.vector.tensor_add(e2[:], e_prev[:], e_sb[:])
                            if kt % 4 == 3:
                                e4 = e4p.tile([128, 1024], bf16, tag="e4")
                                nc.vector.tensor_add(e4[:], e2_prev[:], e2[:])
                                e4s.append(e4)
                        e2_prev = e2
                    e_prev = e_sb
                    if kt == 3 and pending is not None:
                        # deferred: previous iteration's denominator burst +
                        # normalize (and out-proj once both pairs done)
                        p_pr, p_qc, p_e4s, p_ctx = pending
                        with tc.high_priority(offset=400_000):
                            p_dn = psP.tile([128, 512], f32, tag="pp")
                            dn_burst(p_dn, p_e4s, 0, 4, True, True)
                        if p_qc == QC - 1:
                            with tc.high_priority(offset=350_000):
                                finish_iteration(p_pr, p_qc, p_dn, p_ctx)
                        else:
                            finish_iteration(p_pr, p_qc, p_dn, p_ctx)
                        if p_pr == 1:
                            if p_qc >= QC - 2:
                                with tc.high_priority(offset=300_000):
                                    out_proj(p_qc)
                            else:
                                out_proj(p_qc)
                        pending = None
                    if last and kt == 13:
                        # shrink the tail: most of the final denominator
                        # burst can run while the last exps stream
                        dn = psP.tile([128, 512], f32, tag="pp")
                        dn_burst(dn, e4s, 0, 3, True, False)
                if last:
                    dn_burst(dn, e4s, 3, 4, False, True)
                    finish_iteration(pr, qc, dn, ctx_ps)
                    out_proj(qc)
                else:
                    pending = (pr, qc, e4s, ctx_ps)

    nc.compile()
    _NC = nc
    return nc


def prepare_in_maps(inputs):
    q, k, v = inputs["q"], inputs["k"], inputs["v"]
    Wq, bq = inputs["Wq"], inputs["bq"]
    Wk = inputs["Wk"]
    Wv = inputs["Wv"]
    Wo = inputs["Wo"]
    sc = np.float32(1.0 / np.sqrt(D))

    f32, bf = np.float32, ml_dtypes.bfloat16

    def chunk(xT, n, w):
        # [SIZE, S] -> [n, 128, ET, w] with contiguous per-partition lines
        return np.ascontiguousarray(
            xT.reshape(ET, 128, n, w).transpose(2, 1, 0, 3))

    qT = [chunk(q[b].T.astype(bf), QC, 512) for b in range(B)]
    kT = [chunk(k[b].T.astype(bf), QC, 512) for b in range(B)]
    vT = [chunk(v[b].T.astype(bf), KT, 128) for b in range(B)]
    WqTs = (Wq.T * sc).astype(bf)   # attention scale folded into Wq/bq
    WkT = Wk.T.astype(bf)
    WvT = Wv.T.astype(bf)
    WoT = Wo.T.astype(bf)           # [c, o]
    bqs = (bq * sc).astype(f32)

    in_maps = []
    for core in range(NCORES):
        b, hg = divmod(core, HGROUPS)
        sl = slice(hg * D_LOC, (hg + 1) * D_LOC)
        in_maps.append({
            "qTq": qT[b], "kTq": kT[b], "vTq": vT[b],
            "WqT": np.ascontiguousarray(WqTs[:, sl]),
            "WkT": np.ascontiguousarray(WkT[:, sl]),
            "WvT": np.ascontiguousarray(WvT[:, sl]),
            "WoT": np.ascontiguousarray(WoT[sl, :]),
            "bq": np.ascontiguousarray(bqs[sl]),
        })
    return in_maps


def gather(results, inputs):
    # host epilogue: sum the 4 tensor-parallel partials per batch and add the
    # constant row bv @ Wo.T + bo (the value bias commutes through softmax)
    const = (inputs["bv"].astype(np.float64) @ inputs["Wo"].astype(np.float64).T
             + inputs["bo"].astype(np.float64)).astype(np.float32)
    full = np.empty((B, S, SIZE), np.float32)
    for b in range(B):
        acc = results[b * HGROUPS]["out"].astype(np.float32)
        for hg in range(1, HGROUPS):
            acc = acc + results[b * HGROUPS + hg]["out"].astype(np.float32)
        full[b] = acc + const[None, :]
    return full


def kernel(**inputs):
    nc = build()
    in_maps = prepare_in_maps(inputs)
    res = run_bass_kernel_spmd(nc, in_maps, core_ids=list(range(NCORES)), trace=False)
    return gather(res.results, inputs)
